# revision 21
# baseline (speedup 1.0000x reference)
"""Trainium2 Bass kernel for nn_MiniLLMIndexer.

Computes: q = hs @ wq.T, k = hs @ wk.T (per-head reshape), per-head scaled
attention scores, mean over heads, +mask pad, top-1024 indices (descending,
per query row).

Key algebraic fold: mean over heads of per-head dot products equals one
full-width dot product:
    mean_h(q_h . k_h) * scale = (hs@wq.T) . (hs@wk.T) * scale / NH
so scores_mean = qf @ kf.T * (scale/NH), qf/kf: [S, 256]. No per-head work.

Sharding: 4096 query rows split across 8 cores (512 rows each; cores 0-3
batch 0, cores 4-7 batch 1). Each core computes kf for its whole batch
locally -> no collectives.

Top-k: bitonic sort (descending) of each 2048-wide score row carrying
(fp32 value, index). All 4 row-tiles (512 rows) are packed into the free
dimension of the value instructions ([128, 4, 2048] buffers). Index routing
avoids copy_predicated (1x-only): with the 0/1 winner mask m the routed
indices are nia = ib + m*(ia-ib), nib = ia - m*(ia-ib). Work is split
across engines: row-tiles 0-1 carry u16 indices routed on the DVE via
XOR/MULT tensor_tensor ops (2x_1p 16-bit perf mode), while row-tiles 2-3
carry fp32 indices routed on the otherwise-idle GPSIMD engine (which only
implements fp32 add/subtract/mult) — its ~16us/layer hides under the DVE's
~17us/layer. The DVE per layer does: two is_ge masks (u16 + f32), the fp32
max/min over all 4 tiles, and the 4-op u16 XOR routing for tiles 0-1.
Final merge phase only processes the top half. Sorted indices DMA straight
to HBM (u16 tiles 0-1, f32 tiles 2-3); the host casts to int32.
"""

import os
import sys

if "/opt/trn_rl_repo" not in sys.path:
    sys.path.insert(0, "/opt/trn_rl_repo")

import numpy as np

from concourse import bacc, bass, mybir, tile
from concourse.bass_utils import run_bass_kernel_spmd

B, S, HID = 2, 2048, 1024
NH, HD = 8, 32
TOPK = 1024
NCORES = 8
ROWS_PER_CORE = (B * S) // NCORES  # 512
D = NH * HD  # 256
SCALE = (HD ** -0.5) / NH

F32 = mybir.dt.float32
U16 = mybir.dt.uint16
I32 = mybir.dt.int32

_CACHE = {}


def _network_layers(n=S):
    """Bitonic network: descending sort via flip-merge. Returns list of
    (kind, param, width) where width limits processing to the first
    `width` elements (final merge only needs the top half)."""
    layers = []
    m = 1
    while 2 * m <= n:
        layers.append(("flip", m, n))
        d = m // 2
        width = n // 2 if 2 * m == n else n
        while d >= 1:
            layers.append(("dist", d, width))
            d //= 2
        m *= 2
    return layers


def _build_program():
    nc = bacc.Bacc(None, target_bir_lowering=False)

    hsT = nc.dram_tensor("hsT", [HID, S], F32, kind="ExternalInput")
    hsTo = nc.dram_tensor("hsTo", [HID, ROWS_PER_CORE], F32, kind="ExternalInput")
    wqT = nc.dram_tensor("wqT", [HID, D], F32, kind="ExternalInput")
    wkT = nc.dram_tensor("wkT", [HID, D], F32, kind="ExternalInput")
    maskd = nc.dram_tensor("maskd", [1, S], F32, kind="ExternalInput")
    # tiles 0-1 (rows 0-255): u16 indices; tiles 2-3 (rows 256-511): f32
    out01 = nc.dram_tensor("out01", [ROWS_PER_CORE // 2, TOPK], U16,
                           kind="ExternalOutput")
    out23 = nc.dram_tensor("out23", [ROWS_PER_CORE // 2, TOPK], F32,
                           kind="ExternalOutput")

    HC = HID // 128  # 8 contraction chunks
    DC = D // 128    # 2 d-half chunks
    JC = S // 512    # 4 column chunks
    RT = ROWS_PER_CORE // 128  # 4 row tiles
    GT = RT // 2     # 2 tiles per engine group
    HP = GT * (S // 2)  # mask/pair elements per group per partition

    layers = _network_layers()
    # perf-attribution knob; default runs the full network
    _nl = os.environ.get("BASS_SORT_LAYERS")
    if _nl is not None:
        layers = layers[: int(_nl)]

    with tile.TileContext(nc) as tc:
        with (
            tc.tile_pool(name="weights", bufs=1) as wpool,
            tc.tile_pool(name="kf", bufs=1) as kfpool,
            tc.tile_pool(name="psum", bufs=1, space="PSUM") as psum,
            tc.tile_pool(name="small", bufs=1) as small,
            tc.tile_pool(name="stream", bufs=2) as stpool,
            tc.tile_pool(name="sort", bufs=1) as spool,
        ):
            # ---- load weights / mask ----
            wq_sb = wpool.tile([128, HC, D], F32, tag="wq")
            wk_sb = wpool.tile([128, HC, D], F32, tag="wk")
            nc.sync.dma_start(wq_sb[:], wqT.rearrange("(c p) f -> p c f", p=128))
            nc.sync.dma_start(wk_sb[:], wkT.rearrange("(c p) f -> p c f", p=128))

            pad_sb = small.tile([1, S], F32, tag="pad")
            nc.sync.dma_start(pad_sb[:], maskd[:])
            # pad = (1 - mask) * -1e9 = mask*1e9 - 1e9 (in place)
            nc.vector.tensor_scalar(
                pad_sb[:], pad_sb[:], 1e9, scalar2=1e9,
                op0=mybir.AluOpType.mult, op1=mybir.AluOpType.subtract,
            )
            ones_sb = small.tile([1, 128], F32, tag="ones")
            nc.vector.memset(ones_sb[:], 1.0)

            # tiny dummy matmuls so the PE queue observes the weight-DMA
            # semaphores before any real matmul (PE LDW has 1 wait slot)
            dummy_ps = psum.tile([1, 1], F32, tag="kps0")
            nc.tensor.matmul(dummy_ps[:], wq_sb[:, 0, 0:1], wq_sb[:, 0, 0:1])
            nc.tensor.matmul(dummy_ps[:], wk_sb[:, 0, 0:1], wk_sb[:, 0, 0:1])

            # ---- qfT[d, i] (scaled): 2 tiles [128, 512] ----
            qf_sb = wpool.tile([128, DC, ROWS_PER_CORE], F32, tag="qf")
            qf_ps = [psum.tile([128, ROWS_PER_CORE], F32, name=f"qps{dh}",
                               tag=f"kps{dh}") for dh in range(DC)]
            for h in range(HC):
                ch = stpool.tile([128, 2 * ROWS_PER_CORE], F32, tag="hs_ch")
                eng = nc.sync if h % 2 == 0 else nc.scalar
                eng.dma_start(
                    ch[:, :ROWS_PER_CORE],
                    hsTo.rearrange("(c p) f -> p c f", p=128)[:, h, :])
                for dh in range(DC):
                    nc.tensor.matmul(
                        qf_ps[dh][:],
                        wq_sb[:, h, dh * 128:(dh + 1) * 128],
                        ch[:, :ROWS_PER_CORE],
                        start=(h == 0), stop=(h == HC - 1),
                    )
            for dh in range(DC):
                nc.scalar.activation(
                    qf_sb[:, dh, :], qf_ps[dh][:],
                    mybir.ActivationFunctionType.Copy, scale=float(SCALE),
                )

            # ---- kfT[d, j]: single pass over streamed hs chunks ----
            kf_sb = kfpool.tile([128, DC, S], F32, tag="kf")
            kf_ps = [[psum.tile([128, 512], F32, name=f"kps{dh}_{jc}",
                                tag=f"kps{dh * JC + jc}") for jc in range(JC)]
                     for dh in range(DC)]
            for h in range(HC):
                for half in range(2):
                    ch = stpool.tile([128, S // 2], F32, tag="hs_ch")
                    eng = nc.sync if (2 * h + half) % 2 == 0 else nc.scalar
                    eng.dma_start(
                        ch[:],
                        hsT.rearrange("(c p) f -> p c f", p=128)[
                            :, h, half * (S // 2):(half + 1) * (S // 2)])
                    for dh in range(DC):
                        for jl in range(JC // 2):
                            jc = half * (JC // 2) + jl
                            nc.tensor.matmul(
                                kf_ps[dh][jc][:],
                                wk_sb[:, h, dh * 128:(dh + 1) * 128],
                                ch[:, jl * 512:(jl + 1) * 512],
                                start=(h == 0), stop=(h == HC - 1),
                            )
            for dh in range(DC):
                for jc in range(JC):
                    nc.scalar.activation(
                        kf_sb[:, dh, jc * 512:(jc + 1) * 512], kf_ps[dh][jc][:],
                        mybir.ActivationFunctionType.Copy,
                    )

            # ---- scores for all 4 row-tiles -> val_a [128, RT, S] ----
            val_a = spool.tile([128, RT, S], F32, tag="val_a")
            val_b = spool.tile([128, RT, S], F32, tag="val_b")
            i01_a = spool.tile([128, GT, S], U16, tag="i01_a")
            i01_b = spool.tile([128, GT, S], U16, tag="i01_b")
            i23_a = spool.tile([128, GT, S], F32, tag="i23_a")
            i23_b = spool.tile([128, GT, S], F32, tag="i23_b")
            mk01_t = spool.tile([128, HP], U16, tag="mk01")
            # dd doubles as md via an in-place multiply (saves SBUF)
            dd01_t = spool.tile([128, HP], U16, tag="dd01")
            # gpsimd-side mask double-buffered: with a single buffer the
            # DVE's next-layer mask write waits (WAR) on gpsimd's md mult
            # read of the previous layer, which serializes the two engines
            mk23_t = [spool.tile([128, HP], F32, name=f"mk23_{i}",
                                 tag=f"mk23_{i}") for i in range(2)]
            d23_t = spool.tile([128, HP], F32, tag="d23")
            md23_t = spool.tile([128, HP], F32, tag="md23")

            for rt in range(RT):
                for jc in range(JC):
                    acc = psum.tile([128, 512], F32, name=f"sps{rt}_{jc}",
                                    tag=f"kps{(rt * JC + jc) % (DC * JC)}")
                    for dh in range(DC):
                        nc.tensor.matmul(
                            acc[:],
                            qf_sb[:, dh, rt * 128:(rt + 1) * 128],
                            kf_sb[:, dh, jc * 512:(jc + 1) * 512],
                            start=(dh == 0), stop=False,
                        )
                    # + pad broadcast along rows (rank-1 with ones)
                    nc.tensor.matmul(
                        acc[:],
                        ones_sb[:, :],
                        pad_sb[:, jc * 512:(jc + 1) * 512],
                        start=False, stop=True,
                    )
                    nc.scalar.activation(
                        val_a[:, rt, jc * 512:(jc + 1) * 512], acc[:],
                        mybir.ActivationFunctionType.Copy,
                    )

            # index seed: 0..S-1 repeated for each row-tile plane; the f32
            # copy for the gpsimd-routed tiles comes from a converting copy
            nc.gpsimd.iota(i01_a[:], pattern=[[0, GT], [1, S]], base=0,
                           channel_multiplier=0)
            nc.vector.tensor_copy(i23_a[:], i01_a[:])

            cur_v, nxt_v = val_a, val_b
            cur_i0, nxt_i0 = i01_a, i01_b
            cur_i1, nxt_i1 = i23_a, i23_b

            def idx_views(buf, kind, param, width):
                """(ia, ib, nia, nib)-style views of one [128, GT, S] index
                buffer for the given layer geometry."""
                m = param
                if kind == "flip":
                    if width == S:
                        v = buf[:].rearrange(
                            "p t (nb two m) -> p (t nb) two m", two=2, m=m)
                        return v[:, :, 0, :], v[:, :, 1, ::-1]
                    v = buf[:, :, :width].rearrange(
                        "p t (nb two m) -> p t nb two m", two=2, m=m)
                    return v[:, :, :, 0, :], v[:, :, :, 1, ::-1]
                d = param
                if width == S:
                    v = buf[:].rearrange(
                        "p t (nb two d) -> p (t nb) two d", two=2, d=d)
                    return v[:, :, 0, :], v[:, :, 1, :]
                v = buf[:, :, :width].rearrange(
                    "p t (nb two d) -> p t nb two d", two=2, d=d)
                return v[:, :, :, 0, :], v[:, :, :, 1, :]

            def mask_view(buf, kind, param, width):
                m = param
                if width == S:
                    return buf[:].rearrange("p (nb m) -> p nb m", m=m)
                return buf[:, : GT * (width // 2)].rearrange(
                    "p (t nb m) -> p t nb m", t=GT, m=m)

            n_layers = len(layers)
            for li, (kind, param, width) in enumerate(layers):
                last_flip = (kind == "flip" and 2 * param == S)
                first = (li == 0)            # m=1 flip: ia^ib == 1, md == mask
                last = (li == n_layers - 1)  # values dead after final compare
                mk23 = mk23_t[li % 2]

                # per-group value views for the masks (two planes each)
                a0, b0 = idx_views(cur_v[:, 0:GT], kind, param, width)
                a1, b1 = idx_views(cur_v[:, GT:RT], kind, param, width)
                # full 4-tile value views for max/min
                af, bf = idx_views(cur_v, kind, param, width)
                naf, nbf = idx_views(nxt_v, kind, param, width)
                # index views
                ia0, ib0 = idx_views(cur_i0, kind, param, width)
                nia0, nib0 = idx_views(nxt_i0, kind, param, width)
                ia1, ib1 = idx_views(cur_i1, kind, param, width)
                nia1, nib1 = idx_views(nxt_i1, kind, param, width)
                mk0 = mask_view(mk01_t, kind, param, width)
                mk1 = mask_view(mk23, kind, param, width)
                dd0 = mask_view(dd01_t, kind, param, width)
                md0 = dd0
                d1 = mask_view(d23_t, kind, param, width)
                md1 = mask_view(md23_t, kind, param, width)

                # gpsimd group mask first, so the Pool engine starts early
                nc.vector.tensor_tensor(mk1, a1, b1, mybir.AluOpType.is_ge)
                # gpsimd routes f32 indices for tiles 2-3:
                # d = ia-ib; md = d*mask; nia = ib+md; nib = ia-md
                if first:
                    # d == -1, md == -mask
                    nc.gpsimd.tensor_tensor(
                        nia1, ib1, mk1, mybir.AluOpType.subtract)
                    nc.gpsimd.tensor_tensor(
                        nib1, ia1, mk1, mybir.AluOpType.add)
                else:
                    nc.gpsimd.tensor_tensor(
                        d1, ia1, ib1, mybir.AluOpType.subtract)
                    nc.gpsimd.tensor_tensor(md1, d1, mk1, mybir.AluOpType.mult)
                    nc.gpsimd.tensor_tensor(
                        nia1, ib1, md1, mybir.AluOpType.add)
                    if not last_flip:
                        nc.gpsimd.tensor_tensor(
                            nib1, ia1, md1, mybir.AluOpType.subtract)

                # DVE: mask for tiles 0-1, values for all 4 tiles, u16 routing
                nc.vector.tensor_tensor(mk0, a0, b0, mybir.AluOpType.is_ge)
                if not last:
                    nc.vector.tensor_tensor(naf, af, bf, mybir.AluOpType.max)
                    if not last_flip:
                        nc.vector.tensor_tensor(
                            nbf, af, bf, mybir.AluOpType.min)
                # u16 XOR routing: d = ia^ib; md = d*mask; nia = ib^md;
                # nib = ia^md
                if first:
                    md0 = mk0
                else:
                    nc.vector.tensor_tensor(
                        dd0, ia0, ib0, mybir.AluOpType.bitwise_xor)
                    nc.vector.tensor_tensor(
                        md0, dd0, mk0, mybir.AluOpType.mult)
                nc.vector.tensor_tensor(
                    nia0, ib0, md0, mybir.AluOpType.bitwise_xor)
                if not last_flip:
                    nc.vector.tensor_tensor(
                        nib0, ia0, md0, mybir.AluOpType.bitwise_xor)

                cur_v, nxt_v = nxt_v, cur_v
                cur_i0, nxt_i0 = nxt_i0, cur_i0
                cur_i1, nxt_i1 = nxt_i1, cur_i1

            # DMA indices out; host casts to int32
            nc.sync.dma_start(out01.rearrange("(t p) k -> p t k", p=128),
                              cur_i0[:, :, :TOPK])
            nc.sync.dma_start(out23.rearrange("(t p) k -> p t k", p=128),
                              cur_i1[:, :, :TOPK])

    if not nc.is_finalized():
        nc.finalize()
    return nc


def _get_program():
    if "nc" not in _CACHE:
        _CACHE["nc"] = _build_program()
    return _CACHE["nc"]


def kernel(hidden_states, attention_mask, wq, wk, past_len=0):
    hidden_states = np.asarray(hidden_states, dtype=np.float32)
    attention_mask = np.asarray(attention_mask, dtype=np.float32)
    wq = np.asarray(wq, dtype=np.float32)
    wk = np.asarray(wk, dtype=np.float32)

    nc = _get_program()

    wqT = np.ascontiguousarray(wq.T)
    wkT = np.ascontiguousarray(wk.T)
    hsT = [np.ascontiguousarray(hidden_states[b].T) for b in range(B)]

    in_maps = []
    for c in range(NCORES):
        b = c // (NCORES // B)
        r0 = (c % (NCORES // B)) * ROWS_PER_CORE
        in_maps.append({
            "hsT": hsT[b],
            "hsTo": np.ascontiguousarray(hsT[b][:, r0:r0 + ROWS_PER_CORE]),
            "wqT": wqT,
            "wkT": wkT,
            "maskd": attention_mask[b][None, :],
        })

    res = run_bass_kernel_spmd(nc, in_maps, core_ids=list(range(NCORES)))
    parts = []
    for c in range(NCORES):
        lo = res.results[c]["out01"].astype(np.int32)
        hi = res.results[c]["out23"].astype(np.int32)
        parts.append(np.concatenate([lo, hi], axis=0))
    full = np.concatenate(parts, axis=0).reshape(B, S, TOPK)
    return full


# revision 25
# speedup vs baseline: 1.3913x; 1.3913x over previous
"""Trainium2 Bass kernel for nn_MiniLLMIndexer.

Computes: q = hs @ wq.T, k = hs @ wk.T (per-head reshape), per-head scaled
attention scores, mean over heads, +mask pad, top-1024 indices (descending,
per query row).

Key algebraic fold: mean over heads of per-head dot products equals one
full-width dot product:
    mean_h(q_h . k_h) * scale = (hs@wq.T) . (hs@wk.T) * scale / NH
so scores_mean = qf @ kf.T * (scale/NH), qf/kf: [S, 256]. No per-head work.

Sharding: 4096 query rows split across 8 cores (512 rows each; cores 0-3
batch 0, cores 4-7 batch 1). Each core computes kf for its whole batch
locally -> no collectives.

Top-k: bitonic sort (descending) of each 2048-wide score row carrying
(fp32 value, index). All 4 row-tiles (512 rows) are packed into the free
dimension of the value instructions ([128, 4, 2048] buffers). Index routing
avoids copy_predicated (1x-only): with the 0/1 winner mask m the routed
indices are nia = ib + m*(ia-ib), nib = ia - m*(ia-ib). Work is split
across engines: row-tiles 0-1 carry u16 indices routed on the DVE via
XOR/MULT tensor_tensor ops (2x_1p 16-bit perf mode), while row-tiles 2-3
carry fp32 indices routed on the otherwise-idle GPSIMD engine (which only
implements fp32 add/subtract/mult) — its ~16us/layer hides under the DVE's
~17us/layer. The DVE per layer does: two is_ge masks (u16 + f32), the fp32
max/min over all 4 tiles, and the 4-op u16 XOR routing for tiles 0-1.
Final merge phase only processes the top half. Sorted indices DMA straight
to HBM (u16 tiles 0-1, f32 tiles 2-3); the host casts to int32.
"""

import os
import sys

if "/opt/trn_rl_repo" not in sys.path:
    sys.path.insert(0, "/opt/trn_rl_repo")

import numpy as np

from concourse import bacc, bass, mybir, tile
from concourse.bass_utils import run_bass_kernel_spmd

B, S, HID = 2, 2048, 1024
NH, HD = 8, 32
TOPK = 1024
NCORES = 8
ROWS_PER_CORE = (B * S) // NCORES  # 512
D = NH * HD  # 256
SCALE = (HD ** -0.5) / NH

F32 = mybir.dt.float32
F32R = mybir.dt.float32r
U16 = mybir.dt.uint16
I32 = mybir.dt.int32

# fp32r matmuls: 1 cycle/row vs 4 for plain fp32 (moving dim >= 256)
_USE_F32R = os.environ.get("BASS_F32R", "0") == "1"


def _mm(ap):
    return ap.bitcast(F32R) if _USE_F32R else ap

_CACHE = {}


def _network_layers(n=S):
    """Bitonic network: descending sort via flip-merge. Returns list of
    (kind, param, width) where width limits processing to the first
    `width` elements (final merge only needs the top half)."""
    layers = []
    m = 1
    while 2 * m <= n:
        layers.append(("flip", m, n))
        d = m // 2
        width = n // 2 if 2 * m == n else n
        while d >= 1:
            layers.append(("dist", d, width))
            d //= 2
        m *= 2
    return layers


def _build_program():
    nc = bacc.Bacc(None, target_bir_lowering=False)

    hsT = nc.dram_tensor("hsT", [HID, S], F32, kind="ExternalInput")
    hsTo = nc.dram_tensor("hsTo", [HID, ROWS_PER_CORE], F32, kind="ExternalInput")
    wqT = nc.dram_tensor("wqT", [HID, D], F32, kind="ExternalInput")
    wkT = nc.dram_tensor("wkT", [HID, D], F32, kind="ExternalInput")
    maskd = nc.dram_tensor("maskd", [1, S], F32, kind="ExternalInput")
    # tiles 0-1 (rows 0-255): u16 indices; tiles 2-3 (rows 256-511): f32
    out01 = nc.dram_tensor("out01", [ROWS_PER_CORE // 2, TOPK], U16,
                           kind="ExternalOutput")
    out23 = nc.dram_tensor("out23", [ROWS_PER_CORE // 2, TOPK], F32,
                           kind="ExternalOutput")

    HC = HID // 128  # 8 contraction chunks
    DC = D // 128    # 2 d-half chunks
    JC = S // 512    # 4 column chunks
    RT = ROWS_PER_CORE // 128  # 4 row tiles
    GT = RT // 2     # 2 tiles per engine group
    HP = GT * (S // 2)  # mask/pair elements per group per partition

    layers = _network_layers()
    # perf-attribution knob; default runs the full network
    _nl = os.environ.get("BASS_SORT_LAYERS")
    if _nl is not None:
        layers = layers[: int(_nl)]

    with tile.TileContext(nc) as tc:
        with (
            tc.tile_pool(name="weights", bufs=1) as wpool,
            tc.tile_pool(name="kf", bufs=1) as kfpool,
            tc.tile_pool(name="psum", bufs=1, space="PSUM") as psum,
            tc.tile_pool(name="small", bufs=1) as small,
            tc.tile_pool(name="stream", bufs=2) as stpool,
            tc.tile_pool(name="sort", bufs=1) as spool,
        ):
            # ---- load weights / mask ----
            wq_sb = wpool.tile([128, HC, D], F32, tag="wq")
            wk_sb = wpool.tile([128, HC, D], F32, tag="wk")
            nc.sync.dma_start(wq_sb[:], wqT.rearrange("(c p) f -> p c f", p=128))
            nc.sync.dma_start(wk_sb[:], wkT.rearrange("(c p) f -> p c f", p=128))

            pad_sb = small.tile([1, S], F32, tag="pad")
            nc.sync.dma_start(pad_sb[:], maskd[:])
            # pad = (1 - mask) * -1e9 = mask*1e9 - 1e9 (in place)
            nc.vector.tensor_scalar(
                pad_sb[:], pad_sb[:], 1e9, scalar2=1e9,
                op0=mybir.AluOpType.mult, op1=mybir.AluOpType.subtract,
            )
            ones_sb = small.tile([1, 128], F32, tag="ones")
            nc.vector.memset(ones_sb[:], 1.0)

            # tiny dummy matmuls so the PE queue observes the weight-DMA
            # semaphores before any real matmul (PE LDW has 1 wait slot)
            dummy_ps = psum.tile([1, 1], F32, tag="kps0")
            nc.tensor.matmul(dummy_ps[:], wq_sb[:, 0, 0:1], wq_sb[:, 0, 0:1])
            nc.tensor.matmul(dummy_ps[:], wk_sb[:, 0, 0:1], wk_sb[:, 0, 0:1])

            # ---- qfT[d, i] (scaled): 2 tiles [128, 512] ----
            qf_sb = wpool.tile([128, DC, ROWS_PER_CORE], F32, tag="qf")
            qf_ps = [psum.tile([128, ROWS_PER_CORE], F32, name=f"qps{dh}",
                               tag=f"kps{dh}") for dh in range(DC)]
            for h in range(HC):
                ch = stpool.tile([128, 2 * ROWS_PER_CORE], F32, tag="hs_ch")
                eng = nc.sync if h % 2 == 0 else nc.scalar
                eng.dma_start(
                    ch[:, :ROWS_PER_CORE],
                    hsTo.rearrange("(c p) f -> p c f", p=128)[:, h, :])
                for dh in range(DC):
                    nc.tensor.matmul(
                        qf_ps[dh][:],
                        _mm(wq_sb[:, h, dh * 128:(dh + 1) * 128]),
                        _mm(ch[:, :ROWS_PER_CORE]),
                        start=(h == 0), stop=(h == HC - 1),
                    )
            for dh in range(DC):
                nc.scalar.activation(
                    qf_sb[:, dh, :], qf_ps[dh][:],
                    mybir.ActivationFunctionType.Copy, scale=float(SCALE),
                )

            # ---- kfT[d, j]: single pass over streamed hs chunks ----
            kf_sb = kfpool.tile([128, DC, S], F32, tag="kf")
            kf_ps = [[psum.tile([128, 512], F32, name=f"kps{dh}_{jc}",
                                tag=f"kps{dh * JC + jc}") for jc in range(JC)]
                     for dh in range(DC)]
            for h in range(HC):
                for half in range(2):
                    ch = stpool.tile([128, S // 2], F32, tag="hs_ch")
                    eng = nc.sync if (2 * h + half) % 2 == 0 else nc.scalar
                    eng.dma_start(
                        ch[:],
                        hsT.rearrange("(c p) f -> p c f", p=128)[
                            :, h, half * (S // 2):(half + 1) * (S // 2)])
                    for dh in range(DC):
                        for jl in range(JC // 2):
                            jc = half * (JC // 2) + jl
                            nc.tensor.matmul(
                                kf_ps[dh][jc][:],
                                _mm(wk_sb[:, h, dh * 128:(dh + 1) * 128]),
                                _mm(ch[:, jl * 512:(jl + 1) * 512]),
                                start=(h == 0), stop=(h == HC - 1),
                            )
            for dh in range(DC):
                for jc in range(JC):
                    nc.scalar.activation(
                        kf_sb[:, dh, jc * 512:(jc + 1) * 512], kf_ps[dh][jc][:],
                        mybir.ActivationFunctionType.Copy,
                    )

            # ---- scores for all 4 row-tiles -> val_a [128, RT, S] ----
            val_a = spool.tile([128, RT, S], F32, tag="val_a")
            val_b = spool.tile([128, RT, S], F32, tag="val_b")
            i01_a = spool.tile([128, GT, S], U16, tag="i01_a")
            i01_b = spool.tile([128, GT, S], U16, tag="i01_b")
            i23_a = spool.tile([128, GT, S], F32, tag="i23_a")
            i23_b = spool.tile([128, GT, S], F32, tag="i23_b")
            mk01_t = spool.tile([128, HP], U16, tag="mk01")
            # dd doubles as md via an in-place multiply (saves SBUF)
            dd01_t = spool.tile([128, HP], U16, tag="dd01")
            # gpsimd-side mask double-buffered: with a single buffer the
            # DVE's next-layer mask write waits (WAR) on gpsimd's md mult
            # read of the previous layer, which serializes the two engines
            mk23_t = [spool.tile([128, HP], F32, name=f"mk23_{i}",
                                 tag=f"mk23_{i}") for i in range(2)]
            d23_t = spool.tile([128, HP], F32, tag="d23")
            md23_t = spool.tile([128, HP], F32, tag="md23")

            for rt in range(RT):
                for jc in range(JC):
                    acc = psum.tile([128, 512], F32, name=f"sps{rt}_{jc}",
                                    tag=f"kps{(rt * JC + jc) % (DC * JC)}")
                    for dh in range(DC):
                        nc.tensor.matmul(
                            acc[:],
                            _mm(qf_sb[:, dh, rt * 128:(rt + 1) * 128]),
                            _mm(kf_sb[:, dh, jc * 512:(jc + 1) * 512]),
                            start=(dh == 0), stop=False,
                        )
                    # + pad broadcast along rows (rank-1 with ones)
                    nc.tensor.matmul(
                        acc[:],
                        ones_sb[:, :],
                        pad_sb[:, jc * 512:(jc + 1) * 512],
                        start=False, stop=True,
                    )
                    nc.scalar.activation(
                        val_a[:, rt, jc * 512:(jc + 1) * 512], acc[:],
                        mybir.ActivationFunctionType.Copy,
                    )

            # index seed: 0..S-1 repeated for each row-tile plane; the f32
            # copy for the gpsimd-routed tiles comes from a converting copy
            nc.gpsimd.iota(i01_a[:], pattern=[[0, GT], [1, S]], base=0,
                           channel_multiplier=0)
            nc.vector.tensor_copy(i23_a[:], i01_a[:])

            cur_v, nxt_v = val_a, val_b
            cur_i0, nxt_i0 = i01_a, i01_b
            cur_i1, nxt_i1 = i23_a, i23_b

            def idx_views(buf, kind, param, width):
                """(ia, ib, nia, nib)-style views of one [128, GT, S] index
                buffer for the given layer geometry."""
                m = param
                if kind == "flip":
                    if width == S:
                        v = buf[:].rearrange(
                            "p t (nb two m) -> p (t nb) two m", two=2, m=m)
                        return v[:, :, 0, :], v[:, :, 1, ::-1]
                    v = buf[:, :, :width].rearrange(
                        "p t (nb two m) -> p t nb two m", two=2, m=m)
                    return v[:, :, :, 0, :], v[:, :, :, 1, ::-1]
                d = param
                if width == S:
                    v = buf[:].rearrange(
                        "p t (nb two d) -> p (t nb) two d", two=2, d=d)
                    return v[:, :, 0, :], v[:, :, 1, :]
                v = buf[:, :, :width].rearrange(
                    "p t (nb two d) -> p t nb two d", two=2, d=d)
                return v[:, :, :, 0, :], v[:, :, :, 1, :]

            def mask_view(buf, kind, param, width):
                m = param
                if width == S:
                    return buf[:].rearrange("p (nb m) -> p nb m", m=m)
                return buf[:, : GT * (width // 2)].rearrange(
                    "p (t nb m) -> p t nb m", t=GT, m=m)

            n_layers = len(layers)
            for li, (kind, param, width) in enumerate(layers):
                last_flip = (kind == "flip" and 2 * param == S)
                first = (li == 0)            # m=1 flip: ia^ib == 1, md == mask
                last = (li == n_layers - 1)  # values dead after final compare
                mk23 = mk23_t[li % 2]

                # per-group value views for the masks (two planes each)
                a0, b0 = idx_views(cur_v[:, 0:GT], kind, param, width)
                a1, b1 = idx_views(cur_v[:, GT:RT], kind, param, width)
                # full 4-tile value views for max/min
                # index views
                ia0, ib0 = idx_views(cur_i0, kind, param, width)
                nia0, nib0 = idx_views(nxt_i0, kind, param, width)
                ia1, ib1 = idx_views(cur_i1, kind, param, width)
                nia1, nib1 = idx_views(nxt_i1, kind, param, width)
                mk0 = mask_view(mk01_t, kind, param, width)
                mk1 = mask_view(mk23, kind, param, width)
                dd0 = mask_view(dd01_t, kind, param, width)
                md0 = dd0
                d1 = mask_view(d23_t, kind, param, width)
                md1 = mask_view(md23_t, kind, param, width)

                # gpsimd group mask first, so the Pool engine starts early
                nc.vector.tensor_tensor(mk1, a1, b1, mybir.AluOpType.is_ge)
                # gpsimd routes f32 indices for tiles 2-3:
                # d = ia-ib; md = d*mask; nia = ib+md; nib = ia-md
                if first:
                    # d == -1, md == -mask
                    nc.gpsimd.tensor_tensor(
                        nia1, ib1, mk1, mybir.AluOpType.subtract)
                    nc.gpsimd.tensor_tensor(
                        nib1, ia1, mk1, mybir.AluOpType.add)
                else:
                    nc.gpsimd.tensor_tensor(
                        d1, ia1, ib1, mybir.AluOpType.subtract)
                    nc.gpsimd.tensor_tensor(md1, d1, mk1, mybir.AluOpType.mult)
                    nc.gpsimd.tensor_tensor(
                        nia1, ib1, md1, mybir.AluOpType.add)
                    if not last_flip:
                        nc.gpsimd.tensor_tensor(
                            nib1, ia1, md1, mybir.AluOpType.subtract)

                # DVE: mask for tiles 0-1, values for all 4 tiles, u16 routing
                nc.vector.tensor_tensor(mk0, a0, b0, mybir.AluOpType.is_ge)
                if not last:
                    na0, nb0 = idx_views(nxt_v[:, 0:GT], kind, param, width)
                    na1, nb1 = idx_views(nxt_v[:, GT:RT], kind, param, width)
                    nc.vector.tensor_tensor(na0, a0, b0, mybir.AluOpType.max)
                    nc.vector.tensor_tensor(na1, a1, b1, mybir.AluOpType.max)
                    if not last_flip:
                        nc.vector.tensor_tensor(
                            nb0, a0, b0, mybir.AluOpType.min)
                        nc.vector.tensor_tensor(
                            nb1, a1, b1, mybir.AluOpType.min)
                # u16 XOR routing: d = ia^ib; md = d*mask; nia = ib^md;
                # nib = ia^md
                if first:
                    md0 = mk0
                else:
                    nc.vector.tensor_tensor(
                        dd0, ia0, ib0, mybir.AluOpType.bitwise_xor)
                    nc.vector.tensor_tensor(
                        md0, dd0, mk0, mybir.AluOpType.mult)
                nc.vector.tensor_tensor(
                    nia0, ib0, md0, mybir.AluOpType.bitwise_xor)
                if not last_flip:
                    nc.vector.tensor_tensor(
                        nib0, ia0, md0, mybir.AluOpType.bitwise_xor)

                cur_v, nxt_v = nxt_v, cur_v
                cur_i0, nxt_i0 = nxt_i0, cur_i0
                cur_i1, nxt_i1 = nxt_i1, cur_i1

            # DMA indices out; host casts to int32
            nc.sync.dma_start(out01.rearrange("(t p) k -> p t k", p=128),
                              cur_i0[:, :, :TOPK])
            nc.sync.dma_start(out23.rearrange("(t p) k -> p t k", p=128),
                              cur_i1[:, :, :TOPK])

    if not nc.is_finalized():
        nc.finalize()
    return nc


def _get_program():
    if "nc" not in _CACHE:
        _CACHE["nc"] = _build_program()
    return _CACHE["nc"]


def kernel(hidden_states, attention_mask, wq, wk, past_len=0):
    hidden_states = np.asarray(hidden_states, dtype=np.float32)
    attention_mask = np.asarray(attention_mask, dtype=np.float32)
    wq = np.asarray(wq, dtype=np.float32)
    wk = np.asarray(wk, dtype=np.float32)

    nc = _get_program()

    wqT = np.ascontiguousarray(wq.T)
    wkT = np.ascontiguousarray(wk.T)
    hsT = [np.ascontiguousarray(hidden_states[b].T) for b in range(B)]

    in_maps = []
    for c in range(NCORES):
        b = c // (NCORES // B)
        r0 = (c % (NCORES // B)) * ROWS_PER_CORE
        in_maps.append({
            "hsT": hsT[b],
            "hsTo": np.ascontiguousarray(hsT[b][:, r0:r0 + ROWS_PER_CORE]),
            "wqT": wqT,
            "wkT": wkT,
            "maskd": attention_mask[b][None, :],
        })

    res = run_bass_kernel_spmd(nc, in_maps, core_ids=list(range(NCORES)))
    parts = []
    for c in range(NCORES):
        lo = res.results[c]["out01"].astype(np.int32)
        hi = res.results[c]["out23"].astype(np.int32)
        parts.append(np.concatenate([lo, hi], axis=0))
    full = np.concatenate(parts, axis=0).reshape(B, S, TOPK)
    return full
